# revision 32
# baseline (speedup 1.0000x reference)
"""Trainium2 Bass kernel for nn_DecoderBlock (B=32, T=512, D=512, H=8, FFN=2048).

Sharding: data-parallel over batch, 4 batch elements per core across 8 cores,
processed as two pairs; within a pair the two elements are emitted phase-major
so each weight-set load is shared and one element's matmuls fill the other's
pipeline stalls (keeps the PE HAM clock-gate warm at 2.4 GHz).
On-chip layout: activations are feature-major (X.T = [d, t]); all weights are
host-pre-transposed so every matmul operand is a plain contiguous tile. The
bulk data path (X/E/weights/Q/K/V/exp/attn-out/LN-out/FFN) runs in bf16 --
same PE rate as fp32r but half the DMA and SBUF; residual/LN-stat tensors
stay fp32r. Softmax runs without max-subtraction (scores are O(1) scale); the
self-attn mask is added on the PE (identity-matmul accumulate of a
host-precomputed additive bf16 mask, pre-scaled by 8 so the ACT exp's 1/8
scale recovers -1e10); the cross-attn mask is applied by zeroing masked
encoder tokens host-side plus a 0/1 ones-column (emask8), so masked keys drop
out of both the PV sum and the softmax denominator with no exp bias. Softmax
denominators come free from a ones-column appended to V; 1/denom is the
single-op DVE reciprocal_approx_fast (full-tile: the custom ucode needs
partition base 0), broadcast across partitions by a tiny ones-matmul.
LayerNorm stats use ones-matmuls; rstd = exp(-0.5*ln(var+eps)) on ACT so
every activation function used (Exp/Ln/Copy/Square/Lrelu) lives in one act
table family. LN scale/shift is applied via PE outer-products (gamma x rstd,
gamma x mu*rstd - beta). W1 streams per-hc in one strided DMA per chunk; W2
streams per-hc; attention weight sets stage through a 16-buffer pool.
"""
import sys

sys.path.insert(0, '/opt/trn_rl_repo')

import numpy as np

D = 512
T = 512
H = 8
DH = 64
FFN = 2048
B = 32
NCORES = 8
NB = B // NCORES  # batch elements per core
P = 128
NDC = D // P     # 4 feature chunks
NHC = FFN // P   # 16 ffn-hidden chunks
NEG = -1.0e10
EPS = 1e-5

_BUILD_CACHE = {}


def build(nb=NB, reps=1, loop_n=0, act_lrelu=True, dma_lite=False, skip=(),
          approx_recip=True, bcast='pe', lnexp=True):
    key = (nb, reps, loop_n, act_lrelu, dma_lite, tuple(skip),
           approx_recip, bcast, lnexp)
    if key in _BUILD_CACHE:
        return _BUILD_CACHE[key]

    import concourse.bass as bass  # noqa: F401
    import concourse.tile as tile
    import concourse.mybir as mybir
    from concourse import bacc
    from concourse.alu_op_type import AluOpType
    from concourse.masks import make_identity
    from contextlib import ExitStack

    F32 = mybir.dt.float32
    F32R = mybir.dt.float32r
    BF16 = mybir.dt.bfloat16
    AF = mybir.ActivationFunctionType

    def r(ap):
        return ap.bitcast(F32R)

    nc = bacc.Bacc()

    # ---- DRAM I/O (bulk tensors in bf16: halves DMA + SBUF) ----
    xt = nc.dram_tensor("xt", [nb, D, T], BF16, kind="ExternalInput")
    et = nc.dram_tensor("et", [nb, D, T], BF16, kind="ExternalInput")
    maskt = nc.dram_tensor("maskt", [nb, T, T], BF16, kind="ExternalInput")
    # 0/1 ones-column mask for cross-attn V (k >= enc_valid -> 0), replicated
    # 8x per head; enc_outputs itself is zeroed host-side at masked tokens.
    emask8 = nc.dram_tensor("emask8", [nb, P, H * NDC], F32, kind="ExternalInput")
    wname = ["wqt1", "wkt1", "wvt1", "wot1", "wqt2", "wkt2", "wvt2", "wot2"]
    wdr = {n: nc.dram_tensor(n, [D, D], BF16, kind="ExternalInput") for n in wname}
    w1t = nc.dram_tensor("w1t", [D, FFN], BF16, kind="ExternalInput")
    w2t = nc.dram_tensor("w2t", [FFN, D], BF16, kind="ExternalInput")
    b1c = nc.dram_tensor("b1c", [P, NHC], F32, kind="ExternalInput")
    b2c = nc.dram_tensor("b2c", [P, NDC], F32, kind="ExternalInput")
    gb2 = {j: nc.dram_tensor(f"gb2_{j}", [2, D], BF16, kind="ExternalInput")
           for j in (1, 2, 3)}
    ot = nc.dram_tensor("ot", [nb, D, T], BF16, kind="ExternalOutput")

    with tile.TileContext(nc) as tc:
        with ExitStack() as ctx:
            ctx.enter_context(nc.allow_low_precision(
                reason="fp32r is fp32-width; rounding only trims low mantissa bits"))
            # Pin the one act table that serves every function used here
            # (exp/ln/copy/square/parametric_relu all live in set 6,
            # natural_log_exp_and_others). Without this the compiler's
            # table chooser ping-pongs natural_log <-> exp_and_others at
            # every LayerNorm: 24 ACT_TABLE_LOADs x 1.28us, each stalling
            # the ACT queue and starving the PE right at phase boundaries.
            nc.scalar.add_instruction(mybir.InstLoadActFuncSet(
                name=f"I-{nc.scalar.bass.next_id()}", act_func_set_id=6))
            singles = ctx.enter_context(tc.tile_pool(name="singles", bufs=1))
            wa = ctx.enter_context(tc.tile_pool(name="wa", bufs=14))
            w1cp = ctx.enter_context(tc.tile_pool(name="w1c", bufs=16))
            w2p = ctx.enter_context(tc.tile_pool(name="w2", bufs=16))
            xtp = ctx.enter_context(tc.tile_pool(name="xt", bufs=8))
            etp = ctx.enter_context(tc.tile_pool(name="et", bufs=8))
            mtp = ctx.enter_context(tc.tile_pool(name="mt", bufs=4))
            qtp = ctx.enter_context(tc.tile_pool(name="qt", bufs=8))
            ktp = ctx.enter_context(tc.tile_pool(name="kt", bufs=8))
            vop = ctx.enter_context(tc.tile_pool(name="vo", bufs=8))
            exp_pool = ctx.enter_context(tc.tile_pool(name="ex", bufs=6))
            otp = ctx.enter_context(tc.tile_pool(name="otl", bufs=8))
            prelnp = ctx.enter_context(tc.tile_pool(name="preln", bufs=10))
            postlnp = ctx.enter_context(tc.tile_pool(name="postln", bufs=16))
            htp = ctx.enter_context(tc.tile_pool(name="ht", bufs=4))
            smp = ctx.enter_context(tc.tile_pool(name="sm", bufs=6))
            r65p = ctx.enter_context(tc.tile_pool(name="r65", bufs=3))
            stgp = ctx.enter_context(tc.tile_pool(name="stg", bufs=3))
            sqp = ctx.enter_context(tc.tile_pool(name="sq", bufs=3))
            psS = ctx.enter_context(tc.tile_pool(name="psS", bufs=2, space="PSUM"))
            psB = ctx.enter_context(tc.tile_pool(name="psB", bufs=4, space="PSUM"))

            # persistent constants (memset can't write fp32r; stage + rounded copy)
            ones_stage = singles.tile([P, P], F32, tag="ones_stage")
            nc.vector.memset(ones_stage, 1.0)
            ones128 = singles.tile([P, 1], F32, tag="ones128")
            nc.vector.tensor_copy(out=r(ones128), in_=ones_stage[:, 0:1])
            ones65 = singles.tile([65, P], F32, tag="ones65")
            nc.vector.tensor_copy(out=r(ones65), in_=ones_stage[0:65, 0:P])
            onesb = singles.tile([65, P], BF16, tag="onesb")
            nc.vector.tensor_copy(out=onesb, in_=ones_stage[0:65, 0:P])
            eps_t = singles.tile([P, 1], F32, tag="eps")
            nc.vector.memset(eps_t, EPS)
            tb1 = singles.tile([P, NHC], F32, tag="b1")
            nc.sync.dma_start(out=tb1, in_=b1c.ap())
            tb2 = singles.tile([P, NDC], F32, tag="b2")
            nc.sync.dma_start(out=tb2, in_=b2c.ap())
            tgb = {}
            for j in (1, 2, 3):
                tgb[j] = singles.tile([2, D], BF16, tag=f"gb{j}", name=f"gb{j}t")
                nc.sync.dma_start(out=tgb[j], in_=gb2[j].ap())
            # rhs2: row0 = mu*rstd (rewritten per LN), row1 = -1 (constant).
            # Two slots so the two interleaved batch elements' LNs don't
            # serialize on the row-0 rewrite.
            rhs2_stage = singles.tile([2, T], F32, tag="rhs2_stage")
            nc.vector.memset(rhs2_stage, -1.0)
            rhs2s = []
            for s in range(2):
                st = singles.tile([2, T], BF16, tag=f"rhs2_{s}")
                nc.vector.tensor_copy(out=st, in_=rhs2_stage)
                rhs2s.append(st)

            def load_w_tiles(name):
                tiles = []
                for c in range(NDC):
                    t = wa.tile([P, D], BF16, tag="wa")
                    nc.sync.dma_start(out=t,
                                      in_=wdr[name].ap()[c * P:(c + 1) * P, :])
                    tiles.append(t)
                return tiles

            def proj_fm(wtiles, src, out_pool, tag, odt=BF16):
                """out.T[dout,t] = W @ src.T -- all-bf16 operands."""
                outs = []
                for dc in range(NDC):
                    ps = psB.tile([P, T], F32, tag="psB")
                    for kc in range(NDC):
                        nc.tensor.matmul(ps, wtiles[kc][:, dc * P:(dc + 1) * P],
                                         src[kc], start=(kc == 0), stop=(kc == NDC - 1))
                    o = out_pool.tile([P, T], odt, tag=tag)
                    nc.vector.tensor_copy(out=o, in_=ps)
                    outs.append(o)
                return outs

            def proj_vones(wvtiles, src, ones_src=None):
                """Token-major bf16 V with ones columns: vo[kc] = [128(k), 8*65].

                ones_src: [P, H*NDC] 0/1 tile for cross-attn (masked k -> 0);
                None -> plain ones (self-attn)."""
                vos = []
                for kc in range(NDC):
                    ps = psB.tile([P, T], F32, tag="psB")
                    for dcd in range(NDC):
                        nc.tensor.matmul(ps, src[dcd][:, kc * P:(kc + 1) * P],
                                         wvtiles[dcd], start=(dcd == 0), stop=(dcd == NDC - 1))
                    vo = vop.tile([P, H * 65], BF16, tag="vo")
                    osrc = (ones_stage[:, 0:H] if ones_src is None
                            else ones_src[:, kc * H:(kc + 1) * H])
                    nc.any.tensor_copy(
                        out=vo.rearrange("p (h c) -> p h c", c=65)[:, :, 64:65],
                        in_=osrc.rearrange("p (h c) -> p h c", c=1))
                    nc.vector.tensor_copy(
                        out=vo.rearrange("p (h c) -> p h c", c=65)[:, :, 0:64],
                        in_=ps.rearrange("p (h c) -> p h c", c=64))
                    vos.append(vo)
                return vos

            def load_w1_chunk(hc):
                """One hc-column chunk of W1 as [128(d-in-chunk), 4(dc)*128]."""
                t = w1cp.tile([P, NDC * P], BF16, tag="w1c")
                nc.sync.dma_start(
                    out=t.rearrange("p (c f) -> p c f", f=P),
                    in_=w1t.ap().rearrange("(c p) f -> p c f", p=P)
                    [:, :, hc * P:(hc + 1) * P])
                return t

            def attention(els, qt, kt, vo, is_self, mts=None):
                """Multi-head attention for a PAIR of batch elements,
                head-interleaved (e0 h, e1 h, ...) so each element's
                exp/recip dependency chain is hidden behind the other
                element's matmuls. Returns {e: 4 OT tiles [128, T]}."""
                ot_tiles = {e: [otp.tile([P, T], BF16, tag="otl",
                                         name=f"otl{e}_{i}")
                                for i in range(NDC)] for e in els}
                e_tiles = {}

                def scores_exp(e, h):
                    base = (h % 2) * DH
                    cb = h // 2
                    es = []
                    for pair in range(2):
                        sp = psS.tile([P, 2 * T], F32, tag="psS")
                        for half in range(2):
                            kc = pair * 2 + half
                            sl = sp[:, half * T:(half + 1) * T]
                            nc.tensor.matmul(sl,
                                             kt[e][cb][base:base + DH, kc * P:(kc + 1) * P],
                                             qt[e][cb][base:base + DH, :],
                                             start=True, stop=True)
                        ex = exp_pool.tile([P, 2 * T], BF16, tag="ex")
                        nc.scalar.activation(out=ex, in_=sp, func=AF.Exp,
                                             scale=0.125)
                        if is_self:
                            # multiplicative 0/1 mask on the idle GPSIMD
                            # engine: masked k -> e=0, so masked keys drop
                            # out of both the PV sum and the ones-column
                            # denominator (exact, no exp bias).
                            nc.gpsimd.tensor_tensor(out=ex, in0=ex,
                                                    in1=mts[e][pair],
                                                    op=AluOpType.mult)
                        es.append(ex)
                    e_tiles[(e, h)] = es

                def pv_norm(e, h):
                    cb = h // 2
                    es = e_tiles.pop((e, h))
                    pv = psB.tile([65, T], F32, tag="psB")
                    for kc in range(NDC):
                        nc.tensor.matmul(pv, vo[e][kc][:, h * 65:(h + 1) * 65],
                                         es[kc // 2][:, (kc % 2) * T:(kc % 2 + 1) * T],
                                         start=(kc == 0), stop=(kc == NDC - 1))
                    r65 = r65p.tile([65, T], F32, tag="r65")
                    if approx_recip:
                        # full-tile op: custom DVE ucode wants partition base 0;
                        # rows 0..63 are unused junk recips of the PV values
                        nc.vector.reciprocal_approx_fast(out=r65, in_=pv)
                    else:
                        nc.vector.reciprocal(out=r(r65[64:65, :]), in_=pv[64:65, :])
                    rbs = stgp.tile([DH, T], F32, tag="rbs")
                    # bf16 round of the recip row (DVE) feeds a bf16 ones-
                    # matmul broadcast: no fp32r rounding discipline needed,
                    # and 2^-8 rel on the softmax scale is far inside budget
                    rjb = r65p.tile([65, T], BF16, tag="r65b")
                    nc.vector.tensor_copy(out=rjb[64:65, :], in_=r65[64:65, :])
                    rb = psB.tile([P, T], F32, tag="psB")
                    nc.tensor.matmul(rb[0:DH, :], onesb[64:65, 0:DH],
                                     rjb[64:65, :], start=True, stop=True)
                    nc.vector.tensor_copy(out=rbs, in_=rb[0:DH, :])
                    if h % 2 == 0:
                        nc.vector.tensor_tensor(out=ot_tiles[e][cb][0:DH, :],
                                                in0=pv[0:DH, :], in1=rbs,
                                                op=AluOpType.mult)
                    else:
                        stg = stgp.tile([DH, T], BF16, tag="stg")
                        nc.vector.tensor_tensor(out=stg, in0=pv[0:DH, :],
                                                in1=rbs, op=AluOpType.mult)
                        nc.sync.dma_start(out=ot_tiles[e][cb][DH:P, :], in_=stg)

                prev = None
                for h in range(H):
                    for e in els:
                        scores_exp(e, h)
                        if prev is not None:
                            pv_norm(*prev)
                        prev = (e, h)
                pv_norm(*prev)
                return ot_tiles

            def out_proj_residual(wtiles, ot_tiles, resid):
                outs = []
                for dc in range(NDC):
                    ps = psB.tile([P, T], F32, tag="psB")
                    for ic in range(NDC):
                        nc.tensor.matmul(ps, wtiles[ic][:, dc * P:(dc + 1) * P],
                                         ot_tiles[ic], start=(ic == 0), stop=(ic == NDC - 1))
                    o = prelnp.tile([P, T], F32, tag="preln")
                    nc.vector.scalar_tensor_tensor(out=r(o), in0=ps, scalar=1.0,
                                                   in1=resid[dc], op0=AluOpType.mult,
                                                   op1=AluOpType.add)
                    outs.append(o)
                return outs

            def layer_norm(src, j, slot=0, round_out=True, out_views=None):
                """Feature-major layernorm over partition (d) dim.

                Stats via ones-matmuls; scale/shift via PE outer products:
                out = src * (gamma x rstd) - (gamma x mu*rstd - beta)."""
                if 'ln' in skip:
                    return src
                rhs2 = rhs2s[slot]
                s1 = psB.tile([1, T], F32, tag="psB")
                s2 = psB.tile([1, T], F32, tag="psB")
                for dc in range(NDC):
                    nc.tensor.matmul(s1, r(ones128), r(src[dc]),
                                     start=(dc == 0), stop=(dc == NDC - 1))
                for dc in range(NDC):
                    sq = sqp.tile([P, T], F32, tag="sq")
                    nc.vector.tensor_tensor(out=r(sq), in0=src[dc], in1=src[dc],
                                            op=AluOpType.mult)
                    nc.tensor.matmul(s2, r(ones128), r(sq),
                                     start=(dc == 0), stop=(dc == NDC - 1))
                # mu = s1/D and E[x^2] = s2/D via scaled ACT copies; rstd via
                # exp(-0.5*ln(var+eps)) -- same ACT table as Exp/Lrelu/Copy,
                # so the whole kernel runs off one act-table load.
                mu = smp.tile([1, T], F32, tag="sm")
                nc.scalar.activation(out=mu, in_=s1, func=AF.Copy, scale=1.0 / D)
                musq = smp.tile([1, T], F32, tag="sm")
                nc.scalar.activation(out=musq, in_=s1, func=AF.Square,
                                     scale=1.0 / D)
                var = smp.tile([1, T], F32, tag="sm")
                nc.vector.scalar_tensor_tensor(out=var, in0=s2, scalar=1.0 / D,
                                               in1=musq, op0=AluOpType.mult,
                                               op1=AluOpType.subtract)
                rstd = smp.tile([1, T], BF16, tag="smb")
                lv = smp.tile([1, T], F32, tag="sm")
                nc.scalar.activation(out=lv, in_=var, func=AF.Ln,
                                     bias=eps_t[0:1, 0:1])
                nc.scalar.activation(out=rstd, in_=lv, func=AF.Exp,
                                     scale=-0.5)
                nc.vector.tensor_tensor(out=rhs2[0:1, :], in0=mu, in1=rstd,
                                        op=AluOpType.mult)
                outs = []
                for dc in range(NDC):
                    grs = psB.tile([P, T], F32, tag="psB")
                    nc.tensor.matmul(grs, tgb[j][0:1, dc * P:(dc + 1) * P],
                                     rstd, start=True, stop=True)
                    c2 = psB.tile([P, T], F32, tag="psB")
                    nc.tensor.matmul(c2, tgb[j][:, dc * P:(dc + 1) * P],
                                     rhs2, start=True, stop=True)
                    if out_views is not None:
                        o = out_views[dc]
                    elif round_out:
                        o = postlnp.tile([P, T], BF16, tag="postln")
                    else:
                        o = prelnp.tile([P, T], F32, tag="preln")
                    nc.vector.tensor_tensor(out=o, in0=src[dc], in1=grs,
                                            op=AluOpType.mult)
                    nc.vector.tensor_tensor(out=o, in0=o, in1=c2,
                                            op=AluOpType.subtract)
                    outs.append(o)
                return outs

            def ffn_ln3_store(e, zte, slot, w1ts, w2ts):
                """FFN + AddNorm3 + store for one element. W1/W2 chunk tiles
                are loaded once per core (first element's FFN) and stay
                resident for the other three elements -- saves 12MB of HBM
                re-reads. fps holds all 4 psB bufs for the duration (the
                other element is in its LN/store tail)."""
                fps = [psB.tile([P, T], F32, tag="psB", name=f"fps{e}_{i}")
                       for i in range(NDC)]
                h_tiles = {}

                def ffn_h(hc):
                    if hc not in w1ts:
                        w1ts[hc] = load_w1_chunk(hc)
                    w1c_t = w1ts[hc]
                    hp2 = psS.tile([P, 2 * T], F32, tag="psS")
                    hp = hp2[:, 0:T]
                    for dc in range(NDC):
                        nc.tensor.matmul(hp, w1c_t[:, dc * P:(dc + 1) * P],
                                         zte[dc], start=(dc == 0),
                                         stop=(dc == NDC - 1))
                    ht = htp.tile([P, T], BF16, tag="ht")
                    # Prelu == LeakyReLU(alpha) but parametric_relu lives in
                    # every act table incl natural_log_exp -- no table reloads
                    nc.scalar.activation(out=ht, in_=hp, func=AF.Prelu,
                                         bias=tb1[:, hc:hc + 1], scale=1.0,
                                         alpha=0.01)
                    h_tiles[hc] = ht

                def ffn_f(hc):
                    ht = h_tiles.pop(hc)
                    if hc not in w2ts:
                        w2tile = w2p.tile([P, D], BF16, tag="w2")
                        nc.sync.dma_start(out=w2tile,
                                          in_=w2t.ap()[hc * P:(hc + 1) * P, :])
                        w2ts[hc] = w2tile
                    w2tile = w2ts[hc]
                    for dc in range(NDC):
                        nc.tensor.matmul(fps[dc], w2tile[:, dc * P:(dc + 1) * P],
                                         ht, start=(hc == 0), stop=(hc == NHC - 1))

                prevh = None
                for hc in range(NHC):
                    ffn_h(hc)
                    if prevh is not None:
                        ffn_f(prevh)
                    prevh = hc
                ffn_f(prevh)

                out0 = []
                for dc in range(NDC):
                    o = prelnp.tile([P, T], F32, tag="preln")
                    nc.vector.scalar_tensor_tensor(out=r(o), in0=fps[dc],
                                                   scalar=tb2[:, dc:dc + 1],
                                                   in1=zte[dc],
                                                   op0=AluOpType.add,
                                                   op1=AluOpType.add)
                    out0.append(o)
                outt = layer_norm(out0, 3, slot, round_out=True)
                for dc in range(NDC):
                    nc.sync.dma_start(out=ot.ap()[e, dc * P:(dc + 1) * P, :],
                                      in_=outt[dc])

            def body():
              w1ts, w2ts = {}, {}
              for _ in range(reps):
               for ea in range(0, nb, 2):
                es = (ea, ea + 1)
                # ---- load per-element inputs; weights FIRST so the serial
                # SP DMA-issue stream delivers wq1 before the ~12us of
                # mask/em8 issues (masks aren't read until attention) ----
                xts, mts_, em8s, ets = {e: [] for e in es}, {}, {}, {}
                # interleave wq/x chunk loads: the first Q-proj matmul only
                # needs (wq[0], x[0]), so it can start after 2 DMAs, not 8
                wq = []
                for c in range(NDC):
                    t = wa.tile([P, D], BF16, tag="wa")
                    nc.sync.dma_start(out=t,
                                      in_=wdr["wqt1"].ap()[c * P:(c + 1) * P, :])
                    wq.append(t)
                    tx = xtp.tile([P, T], BF16, tag="xt")
                    nc.sync.dma_start(out=tx,
                                      in_=xt.ap()[es[0], c * P:(c + 1) * P, :])
                    xts[es[0]].append(tx)
                for dc in range(NDC):
                    t = xtp.tile([P, T], BF16, tag="xt")
                    nc.sync.dma_start(out=t,
                                      in_=xt.ap()[es[1], dc * P:(dc + 1) * P, :])
                    xts[es[1]].append(t)
                wk = load_w_tiles("wkt1")
                wv = load_w_tiles("wvt1")

                # ---- self attention (weights shared across the pair) ----
                qt = {e: proj_fm(wq, xts[e], qtp, "qt", BF16) for e in es}
                kt = {e: proj_fm(wk, xts[e], ktp, "kt", BF16) for e in es}
                for e in es:
                    mts_[e] = []
                    for pair in range(2):
                        # [P, 2T] pair-layout: matches the exp tile (two
                        # k-chunks side by side along the free dim)
                        t = mtp.tile([P, 2 * T], BF16, tag="mt")
                        nc.sync.dma_start(
                            out=t.rearrange("p (c q) -> p c q", q=T),
                            in_=maskt.ap()[e].rearrange("(c p) q -> p c q", p=P)
                            [:, 2 * pair:2 * pair + 2, :])
                        mts_[e].append(t)
                    em8s[e] = singles.tile([P, H * NDC], F32, tag="sm_eb", name=f"em8_{e}")
                    nc.sync.dma_start(out=em8s[e], in_=emask8.ap()[e])
                vo = {e: proj_vones(wv, xts[e]) for e in es}
                ott = attention(es, qt, kt, vo, True, mts=mts_)
                # issue next-phase weight loads during attention so their
                # data is resident when the LN1 boundary stalls the PE
                wo = load_w_tiles("wot1")
                wq2 = load_w_tiles("wqt2")
                wk2 = load_w_tiles("wkt2")
                wv2 = load_w_tiles("wvt2")
                for e in es:
                    ets[e] = []
                    for dc in range(NDC):
                        t = etp.tile([P, T], BF16, tag="et")
                        nc.sync.dma_start(out=t,
                                          in_=et.ap()[e, dc * P:(dc + 1) * P, :])
                        ets[e].append(t)
                y0 = {e: out_proj_residual(wo, ott[e], xts[e]) for e in es}
                # kt2 depends only on ets/wk2 -- emit between the LNs so the
                # PE has ready matmuls while each LN's rstd chain resolves
                kt2 = {}
                kt2[es[0]] = proj_fm(wk2, ets[es[0]], ktp, "kt", BF16)
                yt = {}
                yt[es[0]] = layer_norm(y0[es[0]], 1, slot=0)
                kt2[es[1]] = proj_fm(wk2, ets[es[1]], ktp, "kt", BF16)
                yt[es[1]] = layer_norm(y0[es[1]], 1, slot=1)

                # ---- cross attention (vo2 first: LN-independent filler) ----
                vo2 = {e: proj_vones(wv2, ets[e], ones_src=em8s[e]) for e in es}
                qt2 = {e: proj_fm(wq2, yt[e], qtp, "qt", BF16) for e in es}
                ot2 = attention(es, qt2, kt2, vo2, False)
                wo2 = load_w_tiles("wot2")
                z0 = {e: out_proj_residual(wo2, ot2[e], yt[e]) for e in es}
                zt = {e: layer_norm(z0[e], 2, slot=i) for i, e in enumerate(es)}

                # ---- FFN + AddNorm3 + store (serial per element: fps needs
                # all 4 psB banks; the other element's LN tail overlaps) ----
                for i, e in enumerate(es):
                    ffn_ln3_store(e, zt[e], i, w1ts, w2ts)

            if loop_n > 1:
                with tc.For_i(0, loop_n, 1):
                    body()
            else:
                body()

    nc.compile()
    _BUILD_CACHE[key] = nc
    return nc


def prep_core_inputs(inputs, nb=NB):
    """Host-side prep: transpose weights/activations, build masks, shard over cores."""
    import ml_dtypes
    BF = ml_dtypes.bfloat16
    X = np.asarray(inputs["X"], np.float32)
    E = np.asarray(inputs["enc_outputs"], np.float32)
    dv = np.asarray(inputs["dec_valid_lens"])
    ev = np.asarray(inputs["enc_valid_lens"])
    pos = np.arange(T)

    shared = {
        "w1t": np.ascontiguousarray(np.asarray(inputs["W1"], np.float32).T).astype(BF),
        "w2t": np.ascontiguousarray(np.asarray(inputs["W2"], np.float32).T).astype(BF),
        "b1c": np.ascontiguousarray(np.asarray(inputs["b1"], np.float32).reshape(NHC, P).T),
        "b2c": np.ascontiguousarray(np.asarray(inputs["b2"], np.float32).reshape(NDC, P).T),
    }
    for j in (1, 2, 3):
        shared[f"gb2_{j}"] = np.ascontiguousarray(np.stack(
            [np.asarray(inputs[f"g{j}"], np.float32),
             np.asarray(inputs[f"be{j}"], np.float32)], axis=0)).astype(BF)
    for n, src in [("wqt1", "Wq1"), ("wkt1", "Wk1"), ("wvt1", "Wv1"), ("wot1", "Wo1"),
                   ("wqt2", "Wq2"), ("wkt2", "Wk2"), ("wvt2", "Wv2"), ("wot2", "Wo2")]:
        shared[n] = np.ascontiguousarray(
            np.asarray(inputs[src], np.float32).T).astype(BF)

    in_maps = []
    ncores = X.shape[0] // nb
    for c in range(ncores):
        sl = slice(c * nb, (c + 1) * nb)
        xtc = np.ascontiguousarray(X[sl].transpose(0, 2, 1)).astype(BF)
        # zero encoder tokens at k >= enc_valid: masked keys then score 0 and
        # masked V rows are 0, so with a 0 ones-column they drop out of both
        # the PV sum and the softmax denominator -- no exp bias needed.
        ev01 = (pos[None, :] < ev[sl][:, None]).astype(np.float32)  # [nb, T]
        etc = np.ascontiguousarray(
            (E[sl] * ev01[:, :, None]).transpose(0, 2, 1)).astype(BF)
        # self mask, multiplicative: maskt[b][k, q] = 1.0 where k < dec_valid[b, q]
        mk = (pos[None, :, None] < dv[sl][:, None, :]).astype(np.float32).astype(BF)
        # emask8[b, p, kc*8+j] = ev01 at k = kc*128 + p (replicated 8x per head)
        em = ev01.reshape(nb, NDC, P).transpose(0, 2, 1)          # [nb, P, NDC]
        em8 = np.ascontiguousarray(np.repeat(em, H, axis=2))      # [nb, P, NDC*8]
        m = {"xt": xtc, "et": etc, "maskt": np.ascontiguousarray(mk), "emask8": em8}
        m.update(shared)
        in_maps.append(m)
    return in_maps


def kernel(**inputs):
    from concourse import bass_utils

    nc = build(NB)
    in_maps = prep_core_inputs(inputs, NB)
    res = bass_utils.run_bass_kernel_spmd(nc, in_maps, core_ids=list(range(NCORES)))
    outs = [r["ot"].transpose(0, 2, 1) for r in res.results]
    return np.ascontiguousarray(np.concatenate(outs, axis=0).astype(np.float32))



# revision 37
# speedup vs baseline: 1.0068x; 1.0068x over previous
"""Trainium2 Bass kernel for nn_DecoderBlock (B=32, T=512, D=512, H=8, FFN=2048).

Sharding: data-parallel over batch, 4 batch elements per core across 8 cores,
processed as two pairs; within a pair the two elements are emitted phase-major
so each weight-set load is shared and one element's matmuls fill the other's
pipeline stalls (keeps the PE HAM clock-gate warm at 2.4 GHz).
On-chip layout: activations are feature-major (X.T = [d, t]); all weights are
host-pre-transposed so every matmul operand is a plain contiguous tile. The
bulk data path (X/E/weights/Q/K/V/exp/attn-out/LN-out/FFN) runs in bf16 --
same PE rate as fp32r but half the DMA and SBUF; residual/LN-stat tensors
stay fp32r. Softmax runs without max-subtraction (scores are O(1) scale); the
self-attn mask is added on the PE (identity-matmul accumulate of a
host-precomputed additive bf16 mask, pre-scaled by 8 so the ACT exp's 1/8
scale recovers -1e10); the cross-attn mask is applied by zeroing masked
encoder tokens host-side plus a 0/1 ones-column (emask8), so masked keys drop
out of both the PV sum and the softmax denominator with no exp bias. Softmax
denominators come free from a ones-column appended to V; 1/denom is the
single-op DVE reciprocal_approx_fast (full-tile: the custom ucode needs
partition base 0), broadcast across partitions by a tiny ones-matmul.
LayerNorm stats use ones-matmuls; rstd = exp(-0.5*ln(var+eps)) on ACT so
every activation function used (Exp/Ln/Copy/Square/Lrelu) lives in one act
table family. LN scale/shift is applied via PE outer-products (gamma x rstd,
gamma x mu*rstd - beta). W1 streams per-hc in one strided DMA per chunk; W2
streams per-hc; attention weight sets stage through a 16-buffer pool.
"""
import sys

sys.path.insert(0, '/opt/trn_rl_repo')

import numpy as np

D = 512
T = 512
H = 8
DH = 64
FFN = 2048
B = 32
NCORES = 8
NB = B // NCORES  # batch elements per core
P = 128
NDC = D // P     # 4 feature chunks
NHC = FFN // P   # 16 ffn-hidden chunks
NEG = -1.0e10
EPS = 1e-5

_BUILD_CACHE = {}


def build(nb=NB, reps=1, loop_n=0, act_lrelu=True, dma_lite=False, skip=(),
          approx_recip=True, bcast='pe', lnexp=True):
    key = (nb, reps, loop_n, act_lrelu, dma_lite, tuple(skip),
           approx_recip, bcast, lnexp)
    if key in _BUILD_CACHE:
        return _BUILD_CACHE[key]

    import concourse.bass as bass  # noqa: F401
    import concourse.tile as tile
    import concourse.mybir as mybir
    from concourse import bacc
    from concourse.alu_op_type import AluOpType
    from concourse.masks import make_identity
    from contextlib import ExitStack

    F32 = mybir.dt.float32
    F32R = mybir.dt.float32r
    BF16 = mybir.dt.bfloat16
    AF = mybir.ActivationFunctionType

    def r(ap):
        return ap.bitcast(F32R)

    nc = bacc.Bacc()

    # ---- DRAM I/O (bulk tensors in bf16: halves DMA + SBUF) ----
    xt = nc.dram_tensor("xt", [nb, D, T], BF16, kind="ExternalInput")
    et = nc.dram_tensor("et", [nb, D, T], BF16, kind="ExternalInput")
    maskt = nc.dram_tensor("maskt", [nb, T, T], BF16, kind="ExternalInput")
    # 0/1 ones-column mask for cross-attn V (k >= enc_valid -> 0), replicated
    # 8x per head; enc_outputs itself is zeroed host-side at masked tokens.
    emask8 = nc.dram_tensor("emask8", [nb, P, H * NDC], F32, kind="ExternalInput")
    wname = ["wqt1", "wkt1", "wvt1", "wot1", "wqt2", "wkt2", "wvt2", "wot2"]
    wdr = {n: nc.dram_tensor(n, [D, D], BF16, kind="ExternalInput") for n in wname}
    w1t = nc.dram_tensor("w1t", [D, FFN], BF16, kind="ExternalInput")
    w2t = nc.dram_tensor("w2t", [FFN, D], BF16, kind="ExternalInput")
    b1c = nc.dram_tensor("b1c", [P, NHC], F32, kind="ExternalInput")
    b2c = nc.dram_tensor("b2c", [P, NDC], F32, kind="ExternalInput")
    gb2 = {j: nc.dram_tensor(f"gb2_{j}", [2, D], BF16, kind="ExternalInput")
           for j in (1, 2, 3)}
    ot = nc.dram_tensor("ot", [nb, D, T], BF16, kind="ExternalOutput")

    with tile.TileContext(nc) as tc:
        with ExitStack() as ctx:
            ctx.enter_context(nc.allow_low_precision(
                reason="fp32r is fp32-width; rounding only trims low mantissa bits"))
            # Pin the one act table that serves every function used here
            # (exp/ln/copy/square/parametric_relu all live in set 6,
            # natural_log_exp_and_others). Without this the compiler's
            # table chooser ping-pongs natural_log <-> exp_and_others at
            # every LayerNorm: 24 ACT_TABLE_LOADs x 1.28us, each stalling
            # the ACT queue and starving the PE right at phase boundaries.
            nc.scalar.add_instruction(mybir.InstLoadActFuncSet(
                name=f"I-{nc.scalar.bass.next_id()}", act_func_set_id=6))
            singles = ctx.enter_context(tc.tile_pool(name="singles", bufs=1))
            wa = ctx.enter_context(tc.tile_pool(name="wa", bufs=14))
            w1cp = ctx.enter_context(tc.tile_pool(name="w1c", bufs=16))
            w2p = ctx.enter_context(tc.tile_pool(name="w2", bufs=16))
            xtp = ctx.enter_context(tc.tile_pool(name="xt", bufs=8))
            etp = ctx.enter_context(tc.tile_pool(name="et", bufs=8))
            mtp = ctx.enter_context(tc.tile_pool(name="mt", bufs=4))
            qtp = ctx.enter_context(tc.tile_pool(name="qt", bufs=8))
            ktp = ctx.enter_context(tc.tile_pool(name="kt", bufs=8))
            vop = ctx.enter_context(tc.tile_pool(name="vo", bufs=8))
            exp_pool = ctx.enter_context(tc.tile_pool(name="ex", bufs=5))
            otp = ctx.enter_context(tc.tile_pool(name="otl", bufs=8))
            prelnp = ctx.enter_context(tc.tile_pool(name="preln", bufs=10))
            postlnp = ctx.enter_context(tc.tile_pool(name="postln", bufs=16))
            htp = ctx.enter_context(tc.tile_pool(name="ht", bufs=4))
            smp = ctx.enter_context(tc.tile_pool(name="sm", bufs=6))
            r65p = ctx.enter_context(tc.tile_pool(name="r65", bufs=3))
            stgp = ctx.enter_context(tc.tile_pool(name="stg", bufs=3))
            sqp = ctx.enter_context(tc.tile_pool(name="sq", bufs=3))
            psS = ctx.enter_context(tc.tile_pool(name="psS", bufs=2, space="PSUM"))
            psB = ctx.enter_context(tc.tile_pool(name="psB", bufs=4, space="PSUM"))

            # persistent constants (memset can't write fp32r; stage + rounded copy)
            ones_stage = singles.tile([P, P], F32, tag="ones_stage")
            nc.vector.memset(ones_stage, 1.0)
            ones128 = singles.tile([P, 1], F32, tag="ones128")
            nc.vector.tensor_copy(out=r(ones128), in_=ones_stage[:, 0:1])
            ones65 = singles.tile([65, P], F32, tag="ones65")
            nc.vector.tensor_copy(out=r(ones65), in_=ones_stage[0:65, 0:P])
            onesb = singles.tile([65, P], BF16, tag="onesb")
            nc.vector.tensor_copy(out=onesb, in_=ones_stage[0:65, 0:P])
            eps_t = singles.tile([P, 1], F32, tag="eps")
            nc.vector.memset(eps_t, EPS)
            tb1 = singles.tile([P, NHC], F32, tag="b1")
            nc.sync.dma_start(out=tb1, in_=b1c.ap())
            tb2 = singles.tile([P, NDC], F32, tag="b2")
            nc.sync.dma_start(out=tb2, in_=b2c.ap())
            tgb = {}
            for j in (1, 2, 3):
                tgb[j] = singles.tile([2, D], BF16, tag=f"gb{j}", name=f"gb{j}t")
                nc.sync.dma_start(out=tgb[j], in_=gb2[j].ap())
            # rhs2: row0 = mu*rstd (rewritten per LN), row1 = -1 (constant).
            # Two slots so the two interleaved batch elements' LNs don't
            # serialize on the row-0 rewrite.
            rhs2_stage = singles.tile([2, T], F32, tag="rhs2_stage")
            nc.vector.memset(rhs2_stage, -1.0)
            rhs2s = []
            for s in range(2):
                st = singles.tile([2, T], BF16, tag=f"rhs2_{s}")
                nc.vector.tensor_copy(out=st, in_=rhs2_stage)
                rhs2s.append(st)

            def load_w_tiles(name):
                tiles = []
                for c in range(NDC):
                    t = wa.tile([P, D], BF16, tag="wa")
                    nc.sync.dma_start(out=t,
                                      in_=wdr[name].ap()[c * P:(c + 1) * P, :])
                    tiles.append(t)
                return tiles

            def proj_fm(wtiles, src, out_pool, tag, odt=BF16):
                """out.T[dout,t] = W @ src.T -- all-bf16 operands."""
                outs = []
                for dc in range(NDC):
                    ps = psB.tile([P, T], F32, tag="psB")
                    for kc in range(NDC):
                        nc.tensor.matmul(ps, wtiles[kc][:, dc * P:(dc + 1) * P],
                                         src[kc], start=(kc == 0), stop=(kc == NDC - 1))
                    o = out_pool.tile([P, T], odt, tag=tag)
                    nc.vector.tensor_copy(out=o, in_=ps)
                    outs.append(o)
                return outs

            def proj_vones(wvtiles, src, ones_src=None):
                """Token-major bf16 V with ones columns: vo[kc] = [128(k), 8*65].

                ones_src: [P, H*NDC] 0/1 tile for cross-attn (masked k -> 0);
                None -> plain ones (self-attn)."""
                vos = []
                for kc in range(NDC):
                    ps = psB.tile([P, T], F32, tag="psB")
                    for dcd in range(NDC):
                        nc.tensor.matmul(ps, src[dcd][:, kc * P:(kc + 1) * P],
                                         wvtiles[dcd], start=(dcd == 0), stop=(dcd == NDC - 1))
                    vo = vop.tile([P, H * 65], BF16, tag="vo")
                    osrc = (ones_stage[:, 0:H] if ones_src is None
                            else ones_src[:, kc * H:(kc + 1) * H])
                    nc.any.tensor_copy(
                        out=vo.rearrange("p (h c) -> p h c", c=65)[:, :, 64:65],
                        in_=osrc.rearrange("p (h c) -> p h c", c=1))
                    nc.vector.tensor_copy(
                        out=vo.rearrange("p (h c) -> p h c", c=65)[:, :, 0:64],
                        in_=ps.rearrange("p (h c) -> p h c", c=64))
                    vos.append(vo)
                return vos

            def load_w1_chunk(hc):
                """One hc-column chunk of W1 as [128(d-in-chunk), 4(dc)*128]."""
                t = w1cp.tile([P, NDC * P], BF16, tag="w1c")
                nc.sync.dma_start(
                    out=t.rearrange("p (c f) -> p c f", f=P),
                    in_=w1t.ap().rearrange("(c p) f -> p c f", p=P)
                    [:, :, hc * P:(hc + 1) * P])
                return t

            def attention(els, qt, kt, vo, is_self, mts=None):
                """Multi-head attention for a PAIR of batch elements,
                head-interleaved (e0 h, e1 h, ...) so each element's
                exp/recip dependency chain is hidden behind the other
                element's matmuls. Returns {e: 4 OT tiles [128, T]}."""
                ot_tiles = {e: [otp.tile([P, T], BF16, tag="otl",
                                         name=f"otl{e}_{i}")
                                for i in range(NDC)] for e in els}
                e_tiles = {}

                def scores_exp(e, h):
                    base = (h % 2) * DH
                    cb = h // 2
                    es = []
                    for pair in range(2):
                        sp = psS.tile([P, 2 * T], F32, tag="psS")
                        for half in range(2):
                            kc = pair * 2 + half
                            sl = sp[:, half * T:(half + 1) * T]
                            nc.tensor.matmul(sl,
                                             kt[e][cb][base:base + DH, kc * P:(kc + 1) * P],
                                             qt[e][cb][base:base + DH, :],
                                             start=True, stop=True)
                        ex = exp_pool.tile([P, 2 * T], BF16, tag="ex")
                        nc.scalar.activation(out=ex, in_=sp, func=AF.Exp,
                                             scale=0.125)
                        if is_self:
                            # multiplicative 0/1 mask on the idle GPSIMD
                            # engine: masked k -> e=0, so masked keys drop
                            # out of both the PV sum and the ones-column
                            # denominator (exact, no exp bias).
                            nc.gpsimd.tensor_tensor(out=ex, in0=ex,
                                                    in1=mts[e][pair],
                                                    op=AluOpType.mult)
                        es.append(ex)
                    e_tiles[(e, h)] = es

                def pv_norm(e, h):
                    cb = h // 2
                    es = e_tiles.pop((e, h))
                    pv = psB.tile([65, T], F32, tag="psB")
                    for kc in range(NDC):
                        nc.tensor.matmul(pv, vo[e][kc][:, h * 65:(h + 1) * 65],
                                         es[kc // 2][:, (kc % 2) * T:(kc % 2 + 1) * T],
                                         start=(kc == 0), stop=(kc == NDC - 1))
                    r65 = r65p.tile([65, T], F32, tag="r65")
                    if approx_recip:
                        # full-tile op: custom DVE ucode wants partition base 0;
                        # rows 0..63 are unused junk recips of the PV values
                        nc.vector.reciprocal_approx_fast(out=r65, in_=pv)
                    else:
                        nc.vector.reciprocal(out=r(r65[64:65, :]), in_=pv[64:65, :])
                    rbs = stgp.tile([DH, T], F32, tag="rbs")
                    # bf16 round of the recip row (DVE) feeds a bf16 ones-
                    # matmul broadcast: no fp32r rounding discipline needed,
                    # and 2^-8 rel on the softmax scale is far inside budget
                    rjb = r65p.tile([65, T], BF16, tag="r65b")
                    nc.vector.tensor_copy(out=rjb[64:65, :], in_=r65[64:65, :])
                    rb = psB.tile([P, T], F32, tag="psB")
                    nc.tensor.matmul(rb[0:DH, :], onesb[64:65, 0:DH],
                                     rjb[64:65, :], start=True, stop=True)
                    nc.vector.tensor_copy(out=rbs, in_=rb[0:DH, :])
                    if h % 2 == 0:
                        nc.vector.tensor_tensor(out=ot_tiles[e][cb][0:DH, :],
                                                in0=pv[0:DH, :], in1=rbs,
                                                op=AluOpType.mult)
                    else:
                        stg = stgp.tile([DH, T], BF16, tag="stg")
                        nc.vector.tensor_tensor(out=stg, in0=pv[0:DH, :],
                                                in1=rbs, op=AluOpType.mult)
                        nc.sync.dma_start(out=ot_tiles[e][cb][DH:P, :], in_=stg)

                prev = None
                for h in range(H):
                    for e in els:
                        scores_exp(e, h)
                        if prev is not None:
                            pv_norm(*prev)
                        prev = (e, h)
                pv_norm(*prev)
                return ot_tiles

            def out_proj_residual(wtiles, ot_tiles, resid):
                outs = []
                for dc in range(NDC):
                    ps = psB.tile([P, T], F32, tag="psB")
                    for ic in range(NDC):
                        nc.tensor.matmul(ps, wtiles[ic][:, dc * P:(dc + 1) * P],
                                         ot_tiles[ic], start=(ic == 0), stop=(ic == NDC - 1))
                    o = prelnp.tile([P, T], F32, tag="preln")
                    nc.vector.scalar_tensor_tensor(out=r(o), in0=ps, scalar=1.0,
                                                   in1=resid[dc], op0=AluOpType.mult,
                                                   op1=AluOpType.add)
                    outs.append(o)
                return outs

            def ln_stats(src, j, slot=0):
                """Stats half of the feature-major layernorm (ones-matmuls
                + ACT/DVE chain). Split from ln_apply so a pair of
                elements' stat chains can be emitted back-to-back: the
                second element's s1/s2 matmuls keep the PE fed while the
                first element's rstd chain resolves."""
                rhs2 = rhs2s[slot]
                s1 = psB.tile([1, T], F32, tag="psB")
                s2 = psB.tile([1, T], F32, tag="psB")
                for dc in range(NDC):
                    nc.tensor.matmul(s1, r(ones128), r(src[dc]),
                                     start=(dc == 0), stop=(dc == NDC - 1))
                for dc in range(NDC):
                    sq = sqp.tile([P, T], F32, tag="sq")
                    nc.vector.tensor_tensor(out=r(sq), in0=src[dc], in1=src[dc],
                                            op=AluOpType.mult)
                    nc.tensor.matmul(s2, r(ones128), r(sq),
                                     start=(dc == 0), stop=(dc == NDC - 1))
                # mu = s1/D and E[x^2] = s2/D via scaled ACT copies; rstd via
                # exp(-0.5*ln(var+eps)) -- same ACT table as Exp/Lrelu/Copy,
                # so the whole kernel runs off one act-table load.
                mu = smp.tile([1, T], F32, tag="sm")
                nc.scalar.activation(out=mu, in_=s1, func=AF.Copy, scale=1.0 / D)
                musq = smp.tile([1, T], F32, tag="sm")
                nc.scalar.activation(out=musq, in_=s1, func=AF.Square,
                                     scale=1.0 / D)
                var = smp.tile([1, T], F32, tag="sm")
                nc.vector.scalar_tensor_tensor(out=var, in0=s2, scalar=1.0 / D,
                                               in1=musq, op0=AluOpType.mult,
                                               op1=AluOpType.subtract)
                rstd = smp.tile([1, T], BF16, tag="smb")
                lv = smp.tile([1, T], F32, tag="sm")
                nc.scalar.activation(out=lv, in_=var, func=AF.Ln,
                                     bias=eps_t[0:1, 0:1])
                nc.scalar.activation(out=rstd, in_=lv, func=AF.Exp,
                                     scale=-0.5)
                nc.vector.tensor_tensor(out=rhs2[0:1, :], in0=mu, in1=rstd,
                                        op=AluOpType.mult)
                return rstd

            def ln_apply(src, j, slot, rstd, round_out=True, out_views=None):
                """Scale/shift half: out = src*(gamma x rstd) -
                (gamma x mu*rstd - beta) via PE outer products."""
                rhs2 = rhs2s[slot]
                outs = []
                for dc in range(NDC):
                    grs = psB.tile([P, T], F32, tag="psB")
                    nc.tensor.matmul(grs, tgb[j][0:1, dc * P:(dc + 1) * P],
                                     rstd, start=True, stop=True)
                    c2 = psB.tile([P, T], F32, tag="psB")
                    nc.tensor.matmul(c2, tgb[j][:, dc * P:(dc + 1) * P],
                                     rhs2, start=True, stop=True)
                    if out_views is not None:
                        o = out_views[dc]
                    elif round_out:
                        o = postlnp.tile([P, T], BF16, tag="postln")
                    else:
                        o = prelnp.tile([P, T], F32, tag="preln")
                    nc.vector.tensor_tensor(out=o, in0=src[dc], in1=grs,
                                            op=AluOpType.mult)
                    nc.vector.tensor_tensor(out=o, in0=o, in1=c2,
                                            op=AluOpType.subtract)
                    outs.append(o)
                return outs

            def layer_norm(src, j, slot=0, round_out=True, out_views=None):
                if 'ln' in skip:
                    return src
                rstd = ln_stats(src, j, slot)
                return ln_apply(src, j, slot, rstd, round_out, out_views)

            def ffn_ln3_store(e, zte, slot, w1ts, w2ts):
                """FFN + AddNorm3 + store for one element. W1/W2 chunk tiles
                are loaded once per core (first element's FFN) and stay
                resident for the other three elements -- saves 12MB of HBM
                re-reads. fps holds all 4 psB bufs for the duration (the
                other element is in its LN/store tail)."""
                fps = [psB.tile([P, T], F32, tag="psB", name=f"fps{e}_{i}")
                       for i in range(NDC)]
                h_tiles = {}

                def ffn_h(hc):
                    if hc not in w1ts:
                        w1ts[hc] = load_w1_chunk(hc)
                    w1c_t = w1ts[hc]
                    hp2 = psS.tile([P, 2 * T], F32, tag="psS")
                    hp = hp2[:, 0:T]
                    for dc in range(NDC):
                        nc.tensor.matmul(hp, w1c_t[:, dc * P:(dc + 1) * P],
                                         zte[dc], start=(dc == 0),
                                         stop=(dc == NDC - 1))
                    ht = htp.tile([P, T], BF16, tag="ht")
                    # Prelu == LeakyReLU(alpha) but parametric_relu lives in
                    # every act table incl natural_log_exp -- no table reloads
                    nc.scalar.activation(out=ht, in_=hp, func=AF.Prelu,
                                         bias=tb1[:, hc:hc + 1], scale=1.0,
                                         alpha=0.01)
                    h_tiles[hc] = ht

                def ffn_f(hc):
                    ht = h_tiles.pop(hc)
                    if hc not in w2ts:
                        w2tile = w2p.tile([P, D], BF16, tag="w2")
                        nc.sync.dma_start(out=w2tile,
                                          in_=w2t.ap()[hc * P:(hc + 1) * P, :])
                        w2ts[hc] = w2tile
                    w2tile = w2ts[hc]
                    for dc in range(NDC):
                        nc.tensor.matmul(fps[dc], w2tile[:, dc * P:(dc + 1) * P],
                                         ht, start=(hc == 0), stop=(hc == NHC - 1))

                prevh = None
                for hc in range(NHC):
                    ffn_h(hc)
                    if prevh is not None:
                        ffn_f(prevh)
                    prevh = hc
                ffn_f(prevh)

                out0 = []
                for dc in range(NDC):
                    o = prelnp.tile([P, T], F32, tag="preln")
                    nc.vector.scalar_tensor_tensor(out=r(o), in0=fps[dc],
                                                   scalar=tb2[:, dc:dc + 1],
                                                   in1=zte[dc],
                                                   op0=AluOpType.add,
                                                   op1=AluOpType.add)
                    out0.append(o)
                outt = layer_norm(out0, 3, slot, round_out=True)
                for dc in range(NDC):
                    nc.sync.dma_start(out=ot.ap()[e, dc * P:(dc + 1) * P, :],
                                      in_=outt[dc])

            def body():
              w1ts, w2ts = {}, {}
              for _ in range(reps):
               for ea in range(0, nb, 2):
                es = (ea, ea + 1)
                # ---- load per-element inputs; weights FIRST so the serial
                # SP DMA-issue stream delivers wq1 before the ~12us of
                # mask/em8 issues (masks aren't read until attention) ----
                xts, mts_, em8s, ets = {e: [] for e in es}, {}, {}, {}
                # interleave wq/x chunk loads: the first Q-proj matmul only
                # needs (wq[0], x[0]), so it can start after 2 DMAs, not 8
                wq = []
                for c in range(NDC):
                    t = wa.tile([P, D], BF16, tag="wa")
                    nc.sync.dma_start(out=t,
                                      in_=wdr["wqt1"].ap()[c * P:(c + 1) * P, :])
                    wq.append(t)
                    tx = xtp.tile([P, T], BF16, tag="xt")
                    nc.sync.dma_start(out=tx,
                                      in_=xt.ap()[es[0], c * P:(c + 1) * P, :])
                    xts[es[0]].append(tx)
                for dc in range(NDC):
                    t = xtp.tile([P, T], BF16, tag="xt")
                    nc.sync.dma_start(out=t,
                                      in_=xt.ap()[es[1], dc * P:(dc + 1) * P, :])
                    xts[es[1]].append(t)
                wk = load_w_tiles("wkt1")
                wv = load_w_tiles("wvt1")

                # ---- self attention (weights shared across the pair) ----
                qt = {e: proj_fm(wq, xts[e], qtp, "qt", BF16) for e in es}
                kt = {e: proj_fm(wk, xts[e], ktp, "kt", BF16) for e in es}
                for e in es:
                    mts_[e] = []
                    for pair in range(2):
                        # [P, 2T] pair-layout: matches the exp tile (two
                        # k-chunks side by side along the free dim)
                        t = mtp.tile([P, 2 * T], BF16, tag="mt")
                        nc.sync.dma_start(
                            out=t.rearrange("p (c q) -> p c q", q=T),
                            in_=maskt.ap()[e].rearrange("(c p) q -> p c q", p=P)
                            [:, 2 * pair:2 * pair + 2, :])
                        mts_[e].append(t)
                    em8s[e] = singles.tile([P, H * NDC], F32, tag="sm_eb", name=f"em8_{e}")
                    nc.sync.dma_start(out=em8s[e], in_=emask8.ap()[e])
                vo = {e: proj_vones(wv, xts[e]) for e in es}
                ott = attention(es, qt, kt, vo, True, mts=mts_)
                # issue next-phase weight loads during attention so their
                # data is resident when the LN1 boundary stalls the PE
                wo = load_w_tiles("wot1")
                wq2 = load_w_tiles("wqt2")
                wk2 = load_w_tiles("wkt2")
                wv2 = load_w_tiles("wvt2")
                for e in es:
                    ets[e] = []
                    for dc in range(NDC):
                        t = etp.tile([P, T], BF16, tag="et")
                        nc.sync.dma_start(out=t,
                                          in_=et.ap()[e, dc * P:(dc + 1) * P, :])
                        ets[e].append(t)
                y0 = {e: out_proj_residual(wo, ott[e], xts[e]) for e in es}
                # kt2 depends only on ets/wk2 -- emit between the LNs so the
                # PE has ready matmuls while each LN's rstd chain resolves
                kt2 = {}
                kt2[es[0]] = proj_fm(wk2, ets[es[0]], ktp, "kt", BF16)
                yt = {}
                yt[es[0]] = layer_norm(y0[es[0]], 1, slot=0)
                kt2[es[1]] = proj_fm(wk2, ets[es[1]], ktp, "kt", BF16)
                yt[es[1]] = layer_norm(y0[es[1]], 1, slot=1)

                # ---- cross attention (vo2 first: LN-independent filler) ----
                vo2 = {e: proj_vones(wv2, ets[e], ones_src=em8s[e]) for e in es}
                qt2 = {e: proj_fm(wq2, yt[e], qtp, "qt", BF16) for e in es}
                ot2 = attention(es, qt2, kt2, vo2, False)
                wo2 = load_w_tiles("wot2")
                z0 = {e: out_proj_residual(wo2, ot2[e], yt[e]) for e in es}
                # paired LN2: both elements' stat matmuls emitted first, so
                # e1's s1/s2 work fills the PE while e0's rstd chain resolves
                rst2 = {e: ln_stats(z0[e], 2, slot=i) for i, e in enumerate(es)}
                zt = {e: ln_apply(z0[e], 2, i, rst2[e]) for i, e in enumerate(es)}

                # ---- FFN + AddNorm3 + store (serial per element: fps needs
                # all 4 psB banks; the other element's LN tail overlaps) ----
                for i, e in enumerate(es):
                    ffn_ln3_store(e, zt[e], i, w1ts, w2ts)

            if loop_n > 1:
                with tc.For_i(0, loop_n, 1):
                    body()
            else:
                body()

    nc.compile()
    _BUILD_CACHE[key] = nc
    return nc


def prep_core_inputs(inputs, nb=NB):
    """Host-side prep: transpose weights/activations, build masks, shard over cores."""
    import ml_dtypes
    BF = ml_dtypes.bfloat16
    X = np.asarray(inputs["X"], np.float32)
    E = np.asarray(inputs["enc_outputs"], np.float32)
    dv = np.asarray(inputs["dec_valid_lens"])
    ev = np.asarray(inputs["enc_valid_lens"])
    pos = np.arange(T)

    shared = {
        "w1t": np.ascontiguousarray(np.asarray(inputs["W1"], np.float32).T).astype(BF),
        "w2t": np.ascontiguousarray(np.asarray(inputs["W2"], np.float32).T).astype(BF),
        "b1c": np.ascontiguousarray(np.asarray(inputs["b1"], np.float32).reshape(NHC, P).T),
        "b2c": np.ascontiguousarray(np.asarray(inputs["b2"], np.float32).reshape(NDC, P).T),
    }
    for j in (1, 2, 3):
        shared[f"gb2_{j}"] = np.ascontiguousarray(np.stack(
            [np.asarray(inputs[f"g{j}"], np.float32),
             np.asarray(inputs[f"be{j}"], np.float32)], axis=0)).astype(BF)
    for n, src in [("wqt1", "Wq1"), ("wkt1", "Wk1"), ("wvt1", "Wv1"), ("wot1", "Wo1"),
                   ("wqt2", "Wq2"), ("wkt2", "Wk2"), ("wvt2", "Wv2"), ("wot2", "Wo2")]:
        shared[n] = np.ascontiguousarray(
            np.asarray(inputs[src], np.float32).T).astype(BF)

    in_maps = []
    ncores = X.shape[0] // nb
    for c in range(ncores):
        sl = slice(c * nb, (c + 1) * nb)
        xtc = np.ascontiguousarray(X[sl].transpose(0, 2, 1)).astype(BF)
        # zero encoder tokens at k >= enc_valid: masked keys then score 0 and
        # masked V rows are 0, so with a 0 ones-column they drop out of both
        # the PV sum and the softmax denominator -- no exp bias needed.
        ev01 = (pos[None, :] < ev[sl][:, None]).astype(np.float32)  # [nb, T]
        etc = np.ascontiguousarray(
            (E[sl] * ev01[:, :, None]).transpose(0, 2, 1)).astype(BF)
        # self mask, multiplicative: maskt[b][k, q] = 1.0 where k < dec_valid[b, q]
        mk = (pos[None, :, None] < dv[sl][:, None, :]).astype(np.float32).astype(BF)
        # emask8[b, p, kc*8+j] = ev01 at k = kc*128 + p (replicated 8x per head)
        em = ev01.reshape(nb, NDC, P).transpose(0, 2, 1)          # [nb, P, NDC]
        em8 = np.ascontiguousarray(np.repeat(em, H, axis=2))      # [nb, P, NDC*8]
        m = {"xt": xtc, "et": etc, "maskt": np.ascontiguousarray(mk), "emask8": em8}
        m.update(shared)
        in_maps.append(m)
    return in_maps


def kernel(**inputs):
    from concourse import bass_utils

    nc = build(NB)
    in_maps = prep_core_inputs(inputs, NB)
    res = bass_utils.run_bass_kernel_spmd(nc, in_maps, core_ids=list(range(NCORES)))
    outs = [r["ot"].transpose(0, 2, 1) for r in res.results]
    return np.ascontiguousarray(np.concatenate(outs, axis=0).astype(np.float32))



# revision 39
# speedup vs baseline: 1.0110x; 1.0041x over previous
"""Trainium2 Bass kernel for nn_DecoderBlock (B=32, T=512, D=512, H=8, FFN=2048).

Sharding: data-parallel over batch, 4 batch elements per core across 8 cores,
processed as two pairs; within a pair the two elements are emitted phase-major
so each weight-set load is shared and one element's matmuls fill the other's
pipeline stalls (keeps the PE HAM clock-gate warm at 2.4 GHz).
On-chip layout: activations are feature-major (X.T = [d, t]); all weights are
host-pre-transposed so every matmul operand is a plain contiguous tile. The
bulk data path (X/E/weights/Q/K/V/exp/attn-out/LN-out/FFN) runs in bf16 --
same PE rate as fp32r but half the DMA and SBUF; residual/LN-stat tensors
stay fp32r. Softmax runs without max-subtraction (scores are O(1) scale); the
self-attn mask is added on the PE (identity-matmul accumulate of a
host-precomputed additive bf16 mask, pre-scaled by 8 so the ACT exp's 1/8
scale recovers -1e10); the cross-attn mask is applied by zeroing masked
encoder tokens host-side plus a 0/1 ones-column (emask8), so masked keys drop
out of both the PV sum and the softmax denominator with no exp bias. Softmax
denominators come free from a ones-column appended to V; 1/denom is the
single-op DVE reciprocal_approx_fast (full-tile: the custom ucode needs
partition base 0), broadcast across partitions by a tiny ones-matmul.
LayerNorm stats use ones-matmuls; rstd = exp(-0.5*ln(var+eps)) on ACT so
every activation function used (Exp/Ln/Copy/Square/Lrelu) lives in one act
table family. LN scale/shift is applied via PE outer-products (gamma x rstd,
gamma x mu*rstd - beta). W1 streams per-hc in one strided DMA per chunk; W2
streams per-hc; attention weight sets stage through a 16-buffer pool.
"""
import sys

sys.path.insert(0, '/opt/trn_rl_repo')

import numpy as np

D = 512
T = 512
H = 8
DH = 64
FFN = 2048
B = 32
NCORES = 8
NB = B // NCORES  # batch elements per core
P = 128
NDC = D // P     # 4 feature chunks
NHC = FFN // P   # 16 ffn-hidden chunks
NEG = -1.0e10
EPS = 1e-5

_BUILD_CACHE = {}


def build(nb=NB, reps=1, loop_n=0, act_lrelu=True, dma_lite=False, skip=(),
          approx_recip=True, bcast='pe', lnexp=True):
    key = (nb, reps, loop_n, act_lrelu, dma_lite, tuple(skip),
           approx_recip, bcast, lnexp)
    if key in _BUILD_CACHE:
        return _BUILD_CACHE[key]

    import concourse.bass as bass  # noqa: F401
    import concourse.tile as tile
    import concourse.mybir as mybir
    from concourse import bacc
    from concourse.alu_op_type import AluOpType
    from concourse.masks import make_identity
    from contextlib import ExitStack

    F32 = mybir.dt.float32
    F32R = mybir.dt.float32r
    BF16 = mybir.dt.bfloat16
    AF = mybir.ActivationFunctionType

    def r(ap):
        return ap.bitcast(F32R)

    nc = bacc.Bacc()

    # ---- DRAM I/O (bulk tensors in bf16: halves DMA + SBUF) ----
    xt = nc.dram_tensor("xt", [nb, D, T], BF16, kind="ExternalInput")
    et = nc.dram_tensor("et", [nb, D, T], BF16, kind="ExternalInput")
    maskt = nc.dram_tensor("maskt", [nb, T, T], BF16, kind="ExternalInput")
    # 0/1 ones-column mask for cross-attn V (k >= enc_valid -> 0), replicated
    # 8x per head; enc_outputs itself is zeroed host-side at masked tokens.
    emask8 = nc.dram_tensor("emask8", [nb, P, H * NDC], F32, kind="ExternalInput")
    wname = ["wqt1", "wkt1", "wvt1", "wot1", "wqt2", "wkt2", "wvt2", "wot2"]
    wdr = {n: nc.dram_tensor(n, [D, D], BF16, kind="ExternalInput") for n in wname}
    w1t = nc.dram_tensor("w1t", [D, FFN], BF16, kind="ExternalInput")
    w2t = nc.dram_tensor("w2t", [FFN, D], BF16, kind="ExternalInput")
    b1c = nc.dram_tensor("b1c", [P, NHC], F32, kind="ExternalInput")
    b2c = nc.dram_tensor("b2c", [P, NDC], F32, kind="ExternalInput")
    gb2 = {j: nc.dram_tensor(f"gb2_{j}", [2, D], BF16, kind="ExternalInput")
           for j in (1, 2, 3)}
    ot = nc.dram_tensor("ot", [nb, D, T], BF16, kind="ExternalOutput")

    with tile.TileContext(nc) as tc:
        with ExitStack() as ctx:
            ctx.enter_context(nc.allow_low_precision(
                reason="fp32r is fp32-width; rounding only trims low mantissa bits"))
            # Pin the one act table that serves every function used here
            # (exp/ln/copy/square/parametric_relu all live in set 6,
            # natural_log_exp_and_others). Without this the compiler's
            # table chooser ping-pongs natural_log <-> exp_and_others at
            # every LayerNorm: 24 ACT_TABLE_LOADs x 1.28us, each stalling
            # the ACT queue and starving the PE right at phase boundaries.
            nc.scalar.add_instruction(mybir.InstLoadActFuncSet(
                name=f"I-{nc.scalar.bass.next_id()}", act_func_set_id=6))
            singles = ctx.enter_context(tc.tile_pool(name="singles", bufs=1))
            wa = ctx.enter_context(tc.tile_pool(name="wa", bufs=14))
            w1cp = ctx.enter_context(tc.tile_pool(name="w1c", bufs=16))
            w2p = ctx.enter_context(tc.tile_pool(name="w2", bufs=16))
            xtp = ctx.enter_context(tc.tile_pool(name="xt", bufs=8))
            etp = ctx.enter_context(tc.tile_pool(name="et", bufs=8))
            mtp = ctx.enter_context(tc.tile_pool(name="mt", bufs=6))
            qtp = ctx.enter_context(tc.tile_pool(name="qt", bufs=8))
            ktp = ctx.enter_context(tc.tile_pool(name="kt", bufs=8))
            vop = ctx.enter_context(tc.tile_pool(name="vo", bufs=8))
            exp_pool = ctx.enter_context(tc.tile_pool(name="ex", bufs=5))
            otp = ctx.enter_context(tc.tile_pool(name="otl", bufs=8))
            prelnp = ctx.enter_context(tc.tile_pool(name="preln", bufs=10))
            postlnp = ctx.enter_context(tc.tile_pool(name="postln", bufs=16))
            htp = ctx.enter_context(tc.tile_pool(name="ht", bufs=4))
            smp = ctx.enter_context(tc.tile_pool(name="sm", bufs=6))
            r65p = ctx.enter_context(tc.tile_pool(name="r65", bufs=3))
            stgp = ctx.enter_context(tc.tile_pool(name="stg", bufs=3))
            sqp = ctx.enter_context(tc.tile_pool(name="sq", bufs=3))
            psS = ctx.enter_context(tc.tile_pool(name="psS", bufs=2, space="PSUM"))
            psB = ctx.enter_context(tc.tile_pool(name="psB", bufs=4, space="PSUM"))

            # persistent constants (memset can't write fp32r; stage + rounded copy)
            ones_stage = singles.tile([P, P], F32, tag="ones_stage")
            nc.vector.memset(ones_stage, 1.0)
            ones128 = singles.tile([P, 1], F32, tag="ones128")
            nc.vector.tensor_copy(out=r(ones128), in_=ones_stage[:, 0:1])
            ones65 = singles.tile([65, P], F32, tag="ones65")
            nc.vector.tensor_copy(out=r(ones65), in_=ones_stage[0:65, 0:P])
            onesb = singles.tile([65, P], BF16, tag="onesb")
            nc.vector.tensor_copy(out=onesb, in_=ones_stage[0:65, 0:P])
            eps_t = singles.tile([P, 1], F32, tag="eps")
            nc.vector.memset(eps_t, EPS)
            tb1 = singles.tile([P, NHC], F32, tag="b1")
            nc.sync.dma_start(out=tb1, in_=b1c.ap())
            tb2 = singles.tile([P, NDC], F32, tag="b2")
            nc.sync.dma_start(out=tb2, in_=b2c.ap())
            tgb = {}
            for j in (1, 2, 3):
                tgb[j] = singles.tile([2, D], BF16, tag=f"gb{j}", name=f"gb{j}t")
                nc.sync.dma_start(out=tgb[j], in_=gb2[j].ap())
            # rhs2: row0 = mu*rstd (rewritten per LN), row1 = -1 (constant).
            # Two slots so the two interleaved batch elements' LNs don't
            # serialize on the row-0 rewrite.
            rhs2_stage = singles.tile([2, T], F32, tag="rhs2_stage")
            nc.vector.memset(rhs2_stage, -1.0)
            rhs2s = []
            for s in range(2):
                st = singles.tile([2, T], BF16, tag=f"rhs2_{s}")
                nc.vector.tensor_copy(out=st, in_=rhs2_stage)
                rhs2s.append(st)

            def load_w_tiles(name):
                tiles = []
                for c in range(NDC):
                    t = wa.tile([P, D], BF16, tag="wa")
                    nc.sync.dma_start(out=t,
                                      in_=wdr[name].ap()[c * P:(c + 1) * P, :])
                    tiles.append(t)
                return tiles

            def proj_fm(wtiles, src, out_pool, tag, odt=BF16):
                """out.T[dout,t] = W @ src.T -- all-bf16 operands."""
                outs = []
                for dc in range(NDC):
                    ps = psB.tile([P, T], F32, tag="psB")
                    for kc in range(NDC):
                        nc.tensor.matmul(ps, wtiles[kc][:, dc * P:(dc + 1) * P],
                                         src[kc], start=(kc == 0), stop=(kc == NDC - 1))
                    o = out_pool.tile([P, T], odt, tag=tag)
                    nc.vector.tensor_copy(out=o, in_=ps)
                    outs.append(o)
                return outs

            def proj_vones(wvtiles, src, ones_src=None):
                """Token-major bf16 V with ones columns: vo[kc] = [128(k), 8*65].

                ones_src: [P, H*NDC] 0/1 tile for cross-attn (masked k -> 0);
                None -> plain ones (self-attn)."""
                vos = []
                for kc in range(NDC):
                    ps = psB.tile([P, T], F32, tag="psB")
                    for dcd in range(NDC):
                        nc.tensor.matmul(ps, src[dcd][:, kc * P:(kc + 1) * P],
                                         wvtiles[dcd], start=(dcd == 0), stop=(dcd == NDC - 1))
                    vo = vop.tile([P, H * 65], BF16, tag="vo")
                    osrc = (ones_stage[:, 0:H] if ones_src is None
                            else ones_src[:, kc * H:(kc + 1) * H])
                    nc.any.tensor_copy(
                        out=vo.rearrange("p (h c) -> p h c", c=65)[:, :, 64:65],
                        in_=osrc.rearrange("p (h c) -> p h c", c=1))
                    nc.vector.tensor_copy(
                        out=vo.rearrange("p (h c) -> p h c", c=65)[:, :, 0:64],
                        in_=ps.rearrange("p (h c) -> p h c", c=64))
                    vos.append(vo)
                return vos

            def load_w1_chunk(hc):
                """One hc-column chunk of W1 as [128(d-in-chunk), 4(dc)*128]."""
                t = w1cp.tile([P, NDC * P], BF16, tag="w1c")
                nc.sync.dma_start(
                    out=t.rearrange("p (c f) -> p c f", f=P),
                    in_=w1t.ap().rearrange("(c p) f -> p c f", p=P)
                    [:, :, hc * P:(hc + 1) * P])
                return t

            def attention(els, qt, kt, vo, is_self, mts=None):
                """Multi-head attention for a PAIR of batch elements,
                head-interleaved (e0 h, e1 h, ...) so each element's
                exp/recip dependency chain is hidden behind the other
                element's matmuls. Returns {e: 4 OT tiles [128, T]}."""
                ot_tiles = {e: [otp.tile([P, T], BF16, tag="otl",
                                         name=f"otl{e}_{i}")
                                for i in range(NDC)] for e in els}
                e_tiles = {}

                def scores_exp(e, h):
                    base = (h % 2) * DH
                    cb = h // 2
                    es = []
                    for pair in range(2):
                        sp = psS.tile([P, 2 * T], F32, tag="psS")
                        for half in range(2):
                            kc = pair * 2 + half
                            sl = sp[:, half * T:(half + 1) * T]
                            nc.tensor.matmul(sl,
                                             kt[e][cb][base:base + DH, kc * P:(kc + 1) * P],
                                             qt[e][cb][base:base + DH, :],
                                             start=True, stop=True)
                        ex = exp_pool.tile([P, 2 * T], BF16, tag="ex")
                        nc.scalar.activation(out=ex, in_=sp, func=AF.Exp,
                                             scale=0.125)
                        if is_self:
                            # multiplicative 0/1 mask on the idle GPSIMD
                            # engine: masked k -> e=0, so masked keys drop
                            # out of both the PV sum and the ones-column
                            # denominator (exact, no exp bias).
                            nc.gpsimd.tensor_tensor(out=ex, in0=ex,
                                                    in1=mts[e][pair],
                                                    op=AluOpType.mult)
                        es.append(ex)
                    e_tiles[(e, h)] = es

                def pv_norm(e, h):
                    cb = h // 2
                    es = e_tiles.pop((e, h))
                    pv = psB.tile([65, T], F32, tag="psB")
                    for kc in range(NDC):
                        nc.tensor.matmul(pv, vo[e][kc][:, h * 65:(h + 1) * 65],
                                         es[kc // 2][:, (kc % 2) * T:(kc % 2 + 1) * T],
                                         start=(kc == 0), stop=(kc == NDC - 1))
                    r65 = r65p.tile([65, T], F32, tag="r65")
                    if approx_recip:
                        # full-tile op: custom DVE ucode wants partition base 0;
                        # rows 0..63 are unused junk recips of the PV values
                        nc.vector.reciprocal_approx_fast(out=r65, in_=pv)
                    else:
                        nc.vector.reciprocal(out=r(r65[64:65, :]), in_=pv[64:65, :])
                    rbs = stgp.tile([DH, T], F32, tag="rbs")
                    # bf16 round of the recip row (DVE) feeds a bf16 ones-
                    # matmul broadcast: no fp32r rounding discipline needed,
                    # and 2^-8 rel on the softmax scale is far inside budget
                    rjb = r65p.tile([65, T], BF16, tag="r65b")
                    nc.vector.tensor_copy(out=rjb[64:65, :], in_=r65[64:65, :])
                    rb = psB.tile([P, T], F32, tag="psB")
                    nc.tensor.matmul(rb[0:DH, :], onesb[64:65, 0:DH],
                                     rjb[64:65, :], start=True, stop=True)
                    nc.vector.tensor_copy(out=rbs, in_=rb[0:DH, :])
                    if h % 2 == 0:
                        nc.vector.tensor_tensor(out=ot_tiles[e][cb][0:DH, :],
                                                in0=pv[0:DH, :], in1=rbs,
                                                op=AluOpType.mult)
                    else:
                        stg = stgp.tile([DH, T], BF16, tag="stg")
                        nc.vector.tensor_tensor(out=stg, in0=pv[0:DH, :],
                                                in1=rbs, op=AluOpType.mult)
                        nc.sync.dma_start(out=ot_tiles[e][cb][DH:P, :], in_=stg)

                prev = None
                for h in range(H):
                    for e in els:
                        scores_exp(e, h)
                        if prev is not None:
                            pv_norm(*prev)
                        prev = (e, h)
                pv_norm(*prev)
                return ot_tiles

            def out_proj_residual(wtiles, ot_tiles, resid):
                outs = []
                for dc in range(NDC):
                    ps = psB.tile([P, T], F32, tag="psB")
                    for ic in range(NDC):
                        nc.tensor.matmul(ps, wtiles[ic][:, dc * P:(dc + 1) * P],
                                         ot_tiles[ic], start=(ic == 0), stop=(ic == NDC - 1))
                    o = prelnp.tile([P, T], F32, tag="preln")
                    nc.vector.scalar_tensor_tensor(out=r(o), in0=ps, scalar=1.0,
                                                   in1=resid[dc], op0=AluOpType.mult,
                                                   op1=AluOpType.add)
                    outs.append(o)
                return outs

            def ln_stats(src, j, slot=0):
                """Stats half of the feature-major layernorm (ones-matmuls
                + ACT/DVE chain). Split from ln_apply so a pair of
                elements' stat chains can be emitted back-to-back: the
                second element's s1/s2 matmuls keep the PE fed while the
                first element's rstd chain resolves."""
                rhs2 = rhs2s[slot]
                s1 = psB.tile([1, T], F32, tag="psB")
                s2 = psB.tile([1, T], F32, tag="psB")
                for dc in range(NDC):
                    nc.tensor.matmul(s1, r(ones128), r(src[dc]),
                                     start=(dc == 0), stop=(dc == NDC - 1))
                for dc in range(NDC):
                    sq = sqp.tile([P, T], F32, tag="sq")
                    nc.vector.tensor_tensor(out=r(sq), in0=src[dc], in1=src[dc],
                                            op=AluOpType.mult)
                    nc.tensor.matmul(s2, r(ones128), r(sq),
                                     start=(dc == 0), stop=(dc == NDC - 1))
                # mu = s1/D and E[x^2] = s2/D via scaled ACT copies; rstd via
                # exp(-0.5*ln(var+eps)) -- same ACT table as Exp/Lrelu/Copy,
                # so the whole kernel runs off one act-table load.
                mu = smp.tile([1, T], F32, tag="sm")
                nc.scalar.activation(out=mu, in_=s1, func=AF.Copy, scale=1.0 / D)
                musq = smp.tile([1, T], F32, tag="sm")
                nc.scalar.activation(out=musq, in_=s1, func=AF.Square,
                                     scale=1.0 / D)
                var = smp.tile([1, T], F32, tag="sm")
                nc.vector.scalar_tensor_tensor(out=var, in0=s2, scalar=1.0 / D,
                                               in1=musq, op0=AluOpType.mult,
                                               op1=AluOpType.subtract)
                rstd = smp.tile([1, T], BF16, tag="smb")
                lv = smp.tile([1, T], F32, tag="sm")
                nc.scalar.activation(out=lv, in_=var, func=AF.Ln,
                                     bias=eps_t[0:1, 0:1])
                nc.scalar.activation(out=rstd, in_=lv, func=AF.Exp,
                                     scale=-0.5)
                nc.vector.tensor_tensor(out=rhs2[0:1, :], in0=mu, in1=rstd,
                                        op=AluOpType.mult)
                return rstd

            def ln_apply(src, j, slot, rstd, round_out=True, out_views=None):
                """Scale/shift half: out = src*(gamma x rstd) -
                (gamma x mu*rstd - beta) via PE outer products."""
                rhs2 = rhs2s[slot]
                outs = []
                for dc in range(NDC):
                    grs = psB.tile([P, T], F32, tag="psB")
                    nc.tensor.matmul(grs, tgb[j][0:1, dc * P:(dc + 1) * P],
                                     rstd, start=True, stop=True)
                    c2 = psB.tile([P, T], F32, tag="psB")
                    nc.tensor.matmul(c2, tgb[j][:, dc * P:(dc + 1) * P],
                                     rhs2, start=True, stop=True)
                    if out_views is not None:
                        o = out_views[dc]
                    elif round_out:
                        o = postlnp.tile([P, T], BF16, tag="postln")
                    else:
                        o = prelnp.tile([P, T], F32, tag="preln")
                    nc.vector.tensor_tensor(out=o, in0=src[dc], in1=grs,
                                            op=AluOpType.mult)
                    nc.vector.tensor_tensor(out=o, in0=o, in1=c2,
                                            op=AluOpType.subtract)
                    outs.append(o)
                return outs

            def layer_norm(src, j, slot=0, round_out=True, out_views=None):
                if 'ln' in skip:
                    return src
                rstd = ln_stats(src, j, slot)
                return ln_apply(src, j, slot, rstd, round_out, out_views)

            def ffn_ln3_store(e, zte, slot, w1ts, w2ts):
                """FFN + AddNorm3 + store for one element. W1/W2 chunk tiles
                are loaded once per core (first element's FFN) and stay
                resident for the other three elements -- saves 12MB of HBM
                re-reads. fps holds all 4 psB bufs for the duration (the
                other element is in its LN/store tail)."""
                fps = [psB.tile([P, T], F32, tag="psB", name=f"fps{e}_{i}")
                       for i in range(NDC)]
                h_tiles = {}

                def ffn_h(hc):
                    if hc not in w1ts:
                        w1ts[hc] = load_w1_chunk(hc)
                    w1c_t = w1ts[hc]
                    hp2 = psS.tile([P, 2 * T], F32, tag="psS")
                    hp = hp2[:, 0:T]
                    for dc in range(NDC):
                        nc.tensor.matmul(hp, w1c_t[:, dc * P:(dc + 1) * P],
                                         zte[dc], start=(dc == 0),
                                         stop=(dc == NDC - 1))
                    ht = htp.tile([P, T], BF16, tag="ht")
                    # Prelu == LeakyReLU(alpha) but parametric_relu lives in
                    # every act table incl natural_log_exp -- no table reloads
                    nc.scalar.activation(out=ht, in_=hp, func=AF.Prelu,
                                         bias=tb1[:, hc:hc + 1], scale=1.0,
                                         alpha=0.01)
                    h_tiles[hc] = ht

                def ffn_f(hc):
                    ht = h_tiles.pop(hc)
                    if hc not in w2ts:
                        w2tile = w2p.tile([P, D], BF16, tag="w2")
                        nc.sync.dma_start(out=w2tile,
                                          in_=w2t.ap()[hc * P:(hc + 1) * P, :])
                        w2ts[hc] = w2tile
                    w2tile = w2ts[hc]
                    for dc in range(NDC):
                        nc.tensor.matmul(fps[dc], w2tile[:, dc * P:(dc + 1) * P],
                                         ht, start=(hc == 0), stop=(hc == NHC - 1))

                prevh = None
                for hc in range(NHC):
                    ffn_h(hc)
                    if prevh is not None:
                        ffn_f(prevh)
                    prevh = hc
                ffn_f(prevh)

                out0 = []
                for dc in range(NDC):
                    o = prelnp.tile([P, T], F32, tag="preln")
                    nc.vector.scalar_tensor_tensor(out=r(o), in0=fps[dc],
                                                   scalar=tb2[:, dc:dc + 1],
                                                   in1=zte[dc],
                                                   op0=AluOpType.add,
                                                   op1=AluOpType.add)
                    out0.append(o)
                outt = layer_norm(out0, 3, slot, round_out=True)
                for dc in range(NDC):
                    nc.sync.dma_start(out=ot.ap()[e, dc * P:(dc + 1) * P, :],
                                      in_=outt[dc])

            def body():
              w1ts, w2ts = {}, {}
              for _ in range(reps):
               for ea in range(0, nb, 2):
                es = (ea, ea + 1)
                # ---- load per-element inputs; weights FIRST so the serial
                # SP DMA-issue stream delivers wq1 before the ~12us of
                # mask/em8 issues (masks aren't read until attention) ----
                xts, mts_, em8s, ets = {e: [] for e in es}, {}, {}, {}
                # interleave wq/x chunk loads: the first Q-proj matmul only
                # needs (wq[0], x[0]), so it can start after 2 DMAs, not 8
                wq = []
                for c in range(NDC):
                    t = wa.tile([P, D], BF16, tag="wa")
                    nc.sync.dma_start(out=t,
                                      in_=wdr["wqt1"].ap()[c * P:(c + 1) * P, :])
                    wq.append(t)
                    tx = xtp.tile([P, T], BF16, tag="xt")
                    nc.sync.dma_start(out=tx,
                                      in_=xt.ap()[es[0], c * P:(c + 1) * P, :])
                    xts[es[0]].append(tx)
                for dc in range(NDC):
                    t = xtp.tile([P, T], BF16, tag="xt")
                    nc.sync.dma_start(out=t,
                                      in_=xt.ap()[es[1], dc * P:(dc + 1) * P, :])
                    xts[es[1]].append(t)
                wk = load_w_tiles("wkt1")
                wv = load_w_tiles("wvt1")

                # ---- self attention (weights shared across the pair) ----
                qt = {e: proj_fm(wq, xts[e], qtp, "qt", BF16) for e in es}
                kt = {e: proj_fm(wk, xts[e], ktp, "kt", BF16) for e in es}
                for e in es:
                    mts_[e] = []
                    for pair in range(2):
                        # [P, 2T] pair-layout: matches the exp tile (two
                        # k-chunks side by side along the free dim)
                        t = mtp.tile([P, 2 * T], BF16, tag="mt")
                        nc.sync.dma_start(
                            out=t.rearrange("p (c q) -> p c q", q=T),
                            in_=maskt.ap()[e].rearrange("(c p) q -> p c q", p=P)
                            [:, 2 * pair:2 * pair + 2, :])
                        mts_[e].append(t)
                    em8s[e] = singles.tile([P, H * NDC], F32, tag="sm_eb", name=f"em8_{e}")
                    nc.sync.dma_start(out=em8s[e], in_=emask8.ap()[e])
                vo = {e: proj_vones(wv, xts[e]) for e in es}
                ott = attention(es, qt, kt, vo, True, mts=mts_)
                # issue next-phase weight loads during attention so their
                # data is resident when the LN1 boundary stalls the PE
                wo = load_w_tiles("wot1")
                wq2 = load_w_tiles("wqt2")
                wk2 = load_w_tiles("wkt2")
                wv2 = load_w_tiles("wvt2")
                for e in es:
                    ets[e] = []
                    for dc in range(NDC):
                        t = etp.tile([P, T], BF16, tag="et")
                        nc.sync.dma_start(out=t,
                                          in_=et.ap()[e, dc * P:(dc + 1) * P, :])
                        ets[e].append(t)
                y0 = {e: out_proj_residual(wo, ott[e], xts[e]) for e in es}
                # kt2 depends only on ets/wk2 -- emit between the LNs so the
                # PE has ready matmuls while each LN's rstd chain resolves
                kt2 = {}
                kt2[es[0]] = proj_fm(wk2, ets[es[0]], ktp, "kt", BF16)
                yt = {}
                yt[es[0]] = layer_norm(y0[es[0]], 1, slot=0)
                kt2[es[1]] = proj_fm(wk2, ets[es[1]], ktp, "kt", BF16)
                yt[es[1]] = layer_norm(y0[es[1]], 1, slot=1)

                # ---- cross attention (vo2 first: LN-independent filler) ----
                vo2 = {e: proj_vones(wv2, ets[e], ones_src=em8s[e]) for e in es}
                qt2 = {e: proj_fm(wq2, yt[e], qtp, "qt", BF16) for e in es}
                ot2 = attention(es, qt2, kt2, vo2, False)
                wo2 = load_w_tiles("wot2")
                z0 = {e: out_proj_residual(wo2, ot2[e], yt[e]) for e in es}
                zt = {e: layer_norm(z0[e], 2, slot=i) for i, e in enumerate(es)}

                # ---- FFN + AddNorm3 + store (serial per element: fps needs
                # all 4 psB banks; the other element's LN tail overlaps) ----
                for i, e in enumerate(es):
                    ffn_ln3_store(e, zt[e], i, w1ts, w2ts)

            if loop_n > 1:
                with tc.For_i(0, loop_n, 1):
                    body()
            else:
                body()

    nc.compile()
    _BUILD_CACHE[key] = nc
    return nc


def prep_core_inputs(inputs, nb=NB):
    """Host-side prep: transpose weights/activations, build masks, shard over cores."""
    import ml_dtypes
    BF = ml_dtypes.bfloat16
    X = np.asarray(inputs["X"], np.float32)
    E = np.asarray(inputs["enc_outputs"], np.float32)
    dv = np.asarray(inputs["dec_valid_lens"])
    ev = np.asarray(inputs["enc_valid_lens"])
    pos = np.arange(T)

    shared = {
        "w1t": np.ascontiguousarray(np.asarray(inputs["W1"], np.float32).T).astype(BF),
        "w2t": np.ascontiguousarray(np.asarray(inputs["W2"], np.float32).T).astype(BF),
        "b1c": np.ascontiguousarray(np.asarray(inputs["b1"], np.float32).reshape(NHC, P).T),
        "b2c": np.ascontiguousarray(np.asarray(inputs["b2"], np.float32).reshape(NDC, P).T),
    }
    for j in (1, 2, 3):
        shared[f"gb2_{j}"] = np.ascontiguousarray(np.stack(
            [np.asarray(inputs[f"g{j}"], np.float32),
             np.asarray(inputs[f"be{j}"], np.float32)], axis=0)).astype(BF)
    for n, src in [("wqt1", "Wq1"), ("wkt1", "Wk1"), ("wvt1", "Wv1"), ("wot1", "Wo1"),
                   ("wqt2", "Wq2"), ("wkt2", "Wk2"), ("wvt2", "Wv2"), ("wot2", "Wo2")]:
        shared[n] = np.ascontiguousarray(
            np.asarray(inputs[src], np.float32).T).astype(BF)

    in_maps = []
    ncores = X.shape[0] // nb
    for c in range(ncores):
        sl = slice(c * nb, (c + 1) * nb)
        xtc = np.ascontiguousarray(X[sl].transpose(0, 2, 1)).astype(BF)
        # zero encoder tokens at k >= enc_valid: masked keys then score 0 and
        # masked V rows are 0, so with a 0 ones-column they drop out of both
        # the PV sum and the softmax denominator -- no exp bias needed.
        ev01 = (pos[None, :] < ev[sl][:, None]).astype(np.float32)  # [nb, T]
        etc = np.ascontiguousarray(
            (E[sl] * ev01[:, :, None]).transpose(0, 2, 1)).astype(BF)
        # self mask, multiplicative: maskt[b][k, q] = 1.0 where k < dec_valid[b, q]
        mk = (pos[None, :, None] < dv[sl][:, None, :]).astype(np.float32).astype(BF)
        # emask8[b, p, kc*8+j] = ev01 at k = kc*128 + p (replicated 8x per head)
        em = ev01.reshape(nb, NDC, P).transpose(0, 2, 1)          # [nb, P, NDC]
        em8 = np.ascontiguousarray(np.repeat(em, H, axis=2))      # [nb, P, NDC*8]
        m = {"xt": xtc, "et": etc, "maskt": np.ascontiguousarray(mk), "emask8": em8}
        m.update(shared)
        in_maps.append(m)
    return in_maps


def kernel(**inputs):
    from concourse import bass_utils

    nc = build(NB)
    in_maps = prep_core_inputs(inputs, NB)
    res = bass_utils.run_bass_kernel_spmd(nc, in_maps, core_ids=list(range(NCORES)))
    outs = [r["ot"].transpose(0, 2, 1) for r in res.results]
    return np.ascontiguousarray(np.concatenate(outs, axis=0).astype(np.float32))

